# revision 7
# baseline (speedup 1.0000x reference)
"""DDSL simplex-FT Bass kernel for Trainium2 (8 NeuronCores), v2.

Math: for triangles (j=2) with vertices P[e,v,:] (from V[E]), densities D,
output spectrum F over the 256x129 rfft2 grid:

  sig_v(e,f)  = 2*pi*(kx*Px_v + ky*Py_v)
  d01=sig0-sig1, d12=sig1-sig2, d20=sig2-sig0,  Q = d01*d12*d20
  tmp_re = -(d12*cos(sig0)+d20*cos(sig1)+d01*cos(sig2))/Q   (etc. for im)
  F_raw  = sum_e CD_e * tmp;  F = -(256^2)*F_raw  (+ DC override)

v2 restructure (vs v1):
  - beta trick: host scales the d-plane coefficients by beta=cd^-1/2, so
    QR3->recip yields R~ = cd*R and G_v = d~_pair * R~ = cd*d_pair*R with
    no separate CD-premultiplied planes (gg matmuls and PSUM arena gone).
  - d01+d12+d20=0  =>  G1 = -(G0+G2), so
    sum_v G_v t_v = G0*(t0-t1) + G2*(t2-t1): no G1 plane; trig DIFF planes
    (Pool) replace the 3-plane multiply.
  - cos from the sin args: cos(x) = sin(pi/2 - |x|), |x|<=pi stays in the
    ACT Sin table range; kills the +0.25 cos matmuls and halves FRAC.
  - products in bf16 (DVE 2x mode), reduction via tensor_scalar accum_out
    in 4x mode; accumulation is fp32 in the accumulator. Host-verified
    numerics: l2 rel err ~7e-3 vs fp64 (gate 2e-2).
  - PSUM per pair: uu 2 banks + dd 2 banks, pool bufs=2 -> full
    pair-level double buffering.

Sharding: frequency rows split 8 ways (32 kx rows x 132 padded ky cols per
core = 33 chunks of 128 freqs on partitions); duplicate elements merged on
host (D aggregated), survivor count padded to n_pad on the free dim.
"""

import math
import numpy as np
import ml_dtypes

N_CORES = 8
N_ELEM = 256
RES0, RES1 = 256, 129
KYPAD = 132  # 32*132 = 4224 = 33*128
ROWS_PER_CORE = 32
CHUNKS = (ROWS_PER_CORE * KYPAD) // 128  # 33
MAGIC = float(np.float32(1.5 * 2**23))
TWO_PI = 2 * math.pi

_compiled = {}


def _split3(v):
    """3-way bf16 split of fp32/64 values: v ~= h+m+l with exact bf16 parts."""
    v32 = np.asarray(v, np.float32)
    h = v32.astype(ml_dtypes.bfloat16)
    r = (v32 - h.astype(np.float32)).astype(np.float32)
    m = r.astype(ml_dtypes.bfloat16)
    l = (r - m.astype(np.float32)).astype(ml_dtypes.bfloat16)
    return h, m, l


def _register_ops():
    import concourse.dve_ops as dve_ops_mod
    from concourse.dve_ops import DveOp, OPS
    from concourse.dve_spec import (
        Spec,
        Src0,
        Src1,
        C0,
        C1,
        One,
        Zero,
        eq,
        select,
        lower as dve_lower,
        _has_src1 as has_src1,
    )
    from concourse.dve_uop import DveOpSpec

    def register_op(name, spec, subdim=False):
        existing = {op.name: op for op in OPS}
        if name in existing:
            return existing[name]
        opcode = dve_ops_mod._CUSTOM_DVE_ROW_BASE + len(OPS)
        assert opcode < 0x20
        dve_ops_mod._SUB_OPCODE_FOR_NAME[name] = opcode
        shas = {}
        for ver in ("v3",):
            uops = dve_lower(spec, ver=ver)
            shas[ver] = DveOpSpec(
                name=name, opcode=opcode, uops=uops, rd1_en=has_src1(spec)
            ).sha(ver)
        op = DveOp(name, spec, subdim=subdim, uops_sha=shas)
        OPS.append(op)
        dve_ops_mod.CUSTOM_DVE_SPECS[name] = spec
        return op

    frac = register_op("FRAC_SCALED", Spec(body=(Src0 - ((Src0 + C0) - C0)) * C1))
    _q = Src0 * Src1 * (Src0 + Src1)
    qr3 = register_op("QR3_GUARD", Spec(body=select(eq(_q, Zero), One, _q)))

    # G = Src1 / Src0 via BITWISE_NOT exponent-flip seed + ONE Newton pass
    # (max rel err 1.7e-3 < the bf16 rounding already in the chain); fusing
    # the divide into the d-plane multiply removes a separate reciprocal op
    # and the Pool-side G0 multiply.
    from concourse.dve_spec import Bin as SBin, AluOp as SAluOp

    def _ref_recip1_mul(in0, in1, c0, c1, c2):
        not_x = (~in0.view(np.int32)).view(np.float32)
        y0 = (not_x * np.float32(c0)).astype(np.float32)
        y1 = (y0 * (np.float32(c1) - in0 * y0)).astype(np.float32)
        return (y1 * in1).astype(np.float32)

    _y0 = SBin(SAluOp.BITWISE_NOT, Src0, Src0) * C0
    _y1 = _y0 * (C1 - Src0 * _y0)
    rcpm = register_op(
        "RECIP1_MUL", Spec(body=_y1 * Src1, reference=_ref_recip1_mul)
    )
    return frac, qr3, rcpm


def _build_program(n_pad):
    import concourse.bacc as bacc
    import concourse.mybir as mybir
    from concourse.tile import TileContext

    FRAC, QR3, RCPM = _register_ops()
    from concourse.dve_ops import RECIP_APPROX_FAST_CONSTS

    RC0 = RECIP_APPROX_FAST_CONSTS["s0"]
    RC1 = RECIP_APPROX_FAST_CONSTS["s1"]

    f32 = mybir.dt.float32
    bf16 = mybir.dt.bfloat16
    nc = bacc.Bacc("TRN2", target_bir_lowering=False)

    lhs_d = nc.dram_tensor("lhs6", [6, CHUNKS * 128], bf16, kind="ExternalInput")
    rhsu_d = nc.dram_tensor("rhsu", [6, 3 * n_pad], bf16, kind="ExternalInput")
    rhsd_d = nc.dram_tensor("rhsd", [6, 2 * n_pad], bf16, kind="ExternalInput")
    fout_d = nc.dram_tensor("fout", [128, 2 * CHUNKS], f32, kind="ExternalOutput")

    E = n_pad
    EB = 3 * E
    Sin = mybir.ActivationFunctionType.Sin
    Abs = mybir.ActivationFunctionType.Abs
    Copy = mybir.ActivationFunctionType.Copy
    mult = mybir.AluOpType.mult
    add = mybir.AluOpType.add
    HB = 512  # psum half stride (cols); one 2KB bank

    # matmul outputs must stay inside one PSUM bank per chunk-half
    assert 3 * E <= HB and 2 * E <= HB, f"bad n_pad {E}"

    with TileContext(nc) as tc:
        with (
            tc.tile_pool(name="const", bufs=1) as cpool,
            tc.tile_pool(name="work", bufs=3) as pool,
            tc.tile_pool(name="psum", bufs=2, space="PSUM") as psp,
        ):
            lhs = cpool.tile([6, CHUNKS * 128], bf16)
            rhsu = cpool.tile([6, 3 * E], bf16)
            rhsd = cpool.tile([6, 2 * E], bf16)
            fout = cpool.tile([128, 2 * CHUNKS], f32)
            pi2 = cpool.tile([128, 1], f32)
            nc.gpsimd.memset(pi2[:], math.pi / 2)
            nc.sync.dma_start(lhs[:], lhs_d[:])
            nc.sync.dma_start(rhsu[:], rhsu_d[:])
            nc.sync.dma_start(rhsd[:], rhsd_d[:])

            # chunks are processed in PSUM-sized pairs, but SBUF elementwise
            # stages run over QUADS (2 pairs) to amortize per-instruction
            # fixed costs (ACT memory-latency bubble, Pool Q7 launch).
            quads = [
                [c for c in range(4 * q, min(4 * q + 4, CHUNKS))]
                for q in range((CHUNKS + 3) // 4)
            ]
            cd = nc.vector._custom_dve
            for chunks in quads:
                Q = len(chunks)
                pairs = [chunks[i : i + 2] for i in range(0, Q, 2)]

                def blk(ap, off, width, stride, lo=0, hi=None):
                    """(128, nblk, width) view of a compact tile."""
                    v = ap.rearrange("p (t x) -> p t x", x=stride)
                    return v[:, lo : (hi if hi is not None else v.shape[1]),
                             off : off + width]

                # quad-wide SBUF tiles
                arg = pool.tile([128, Q * EB], f32, tag="arg")
                d12s = pool.tile([128, Q * E], f32, tag="d12s")
                mQ = pool.tile([128, Q * E], f32, tag="mQ")
                Gt = pool.tile([128, Q * 2 * E], bf16, tag="Gt")

                for ip, pc in enumerate(pairs):
                    T = len(pc)
                    lo, hi = 2 * ip, 2 * ip + T

                    # PSUM per pair: uu = 3 u-planes, dd = [d01|d12] halves
                    uu = psp.tile([128, T * HB], f32, tag="uu")
                    dd = psp.tile([128, T * HB], f32, tag="dd")

                    mm = nc.tensor.matmul
                    for h, c in enumerate(pc):
                        l6 = lhs[:, c * 128 : (c + 1) * 128]
                        b = h * HB
                        for v in range(3):
                            mm(uu[:, b + v * E : b + (v + 1) * E], l6,
                               rhsu[:, v * E : (v + 1) * E],
                               start=True, stop=True)
                        mm(dd[:, b : b + E], l6, rhsd[:, 0:E],
                           start=True, stop=True)
                        mm(dd[:, b + E : b + 2 * E], l6, rhsd[:, E : 2 * E],
                           start=True, stop=True)

                    def pblk(ap, off, width):
                        """(128, T, width) view of a T-halved PSUM arena."""
                        return ap.rearrange("p (t x) -> p t x", x=HB)[
                            :, :, off : off + width
                        ]

                    # d12 PSUM->SBUF (QR3 may keep at most one PSUM operand)
                    nc.scalar.activation(
                        blk(d12s[:], 0, E, E, lo, hi), pblk(dd[:], E, E), Copy
                    )
                    # FRAC: arg = 2*pi*(u - round(u)) in [-pi, pi]
                    cd(FRAC, out=blk(arg[:], 0, EB, EB, lo, hi),
                       in0=pblk(uu[:], 0, EB), s0=MAGIC, s1=TWO_PI)
                    # -Q~ = d01*d12*(d01+d12), zero-guarded
                    cd(QR3, out=blk(mQ[:], 0, E, E, lo, hi),
                       in0=blk(d12s[:], 0, E, E, lo, hi), in1=pblk(dd[:], 0, E))
                    # G planes (bf16): per chunk [G0|G2] = [d12~|d01~]/(-Q~);
                    # one fused seed+NR divide-multiply per plane
                    cd(RCPM, out=blk(Gt[:], 0, E, 2 * E, lo, hi),
                       in0=blk(mQ[:], 0, E, E, lo, hi),
                       in1=blk(d12s[:], 0, E, E, lo, hi), s0=RC0, s1=RC1)
                    cd(RCPM, out=blk(Gt[:], E, E, 2 * E, lo, hi),
                       in0=blk(mQ[:], 0, E, E, lo, hi),
                       in1=pblk(dd[:], 0, E), s0=RC0, s1=RC1)

                # trig planes (bf16): sin(arg); cos(arg) = sin(pi/2 - |arg|)
                tr_s = pool.tile([128, Q * EB], bf16, tag="tr_s")
                nc.scalar.activation(tr_s[:], arg[:], Sin)
                ab = pool.tile([128, Q * EB], f32, tag="ab")
                nc.scalar.activation(ab[:], arg[:], Abs)
                tr_c = pool.tile([128, Q * EB], bf16, tag="tr_c")
                nc.scalar.activation(tr_c[:], ab[:], Sin, bias=pi2[:], scale=-1.0)

                # trig diff planes (bf16, Pool): per chunk [t0-t1 | t2-t1]
                sd = pool.tile([128, Q * 2 * E], bf16, tag="sd")
                cdf = pool.tile([128, Q * 2 * E], bf16, tag="cdf")
                for src, dst in ((tr_s, sd), (tr_c, cdf)):
                    nc.gpsimd.tensor_sub(
                        blk(dst[:], 0, E, 2 * E), blk(src[:], 0, E, EB),
                        blk(src[:], E, E, EB),
                    )
                    nc.gpsimd.tensor_sub(
                        blk(dst[:], E, E, 2 * E), blk(src[:], 2 * E, E, EB),
                        blk(src[:], E, E, EB),
                    )

                # products (bf16, DVE 2x) and per-chunk fp32 accumulation
                # (tensor_scalar 4x with accum_out straight into fout)
                pr_re = pool.tile([128, Q * 2 * E], bf16, tag="pr_re")
                pr_im = pool.tile([128, Q * 2 * E], bf16, tag="pr_im")
                nc.vector.tensor_mul(pr_re[:], Gt[:], cdf[:])
                nc.vector.tensor_mul(pr_im[:], Gt[:], sd[:])
                scr = pool.tile([128, 8 * 2 * E], bf16, tag="scr")
                for h, c in enumerate(chunks):
                    for k, pr in enumerate((pr_re, pr_im)):
                        nc.vector.tensor_scalar(
                            out=scr[:, (2 * h + k) * 2 * E : (2 * h + k + 1) * 2 * E],
                            in0=pr[:, h * 2 * E : (h + 1) * 2 * E],
                            scalar1=1.0, scalar2=0.0, op0=mult, op1=add,
                            accum_out=fout[:, 2 * c + k : 2 * c + k + 1],
                        )

            nc.sync.dma_start(fout_d[:], fout[:])

    nc.compile()
    return nc


def _host_prep_group(P, Dagg, n_pad):
    """Build per-core input maps for one padded element group."""
    n_eff = P.shape[0]
    # pad with copies of element 0 carrying zero density (zero contribution)
    if n_pad > n_eff:
        P = np.concatenate([P, np.repeat(P[:1], n_pad - n_eff, axis=0)], axis=0)
        Dagg = np.concatenate(
            [Dagg, np.zeros((n_pad - n_eff, Dagg.shape[1]))], axis=0
        )
    ne = n_pad

    # CD = 2 * area * D via Cayley-Menger (matches reference up to fp rounding)
    D2 = ((P[:, :, None, :] - P[:, None, :, :]) ** 2).sum(-1)
    B = np.ones((ne, 4, 4))
    B[:, 0, 0] = 0.0
    B[:, 1:, 1:] = D2
    vol2 = (-1.0) / 4.0 * np.linalg.det(B) / 4.0  # ((-1)^3)/(2^2)/(2!^2)*det
    content = np.sqrt(np.clip(vol2, 0.0, None))
    CD = 2.0 * content[:, None] * Dagg  # (ne, n_ch=1)
    cdv = CD[:, 0]  # n_ch == 1

    # beta trick: scale d-plane coefficients by cd^-1/2 so G = d~*R~ carries
    # cd automatically; cd==0 (padding / zero-density) rows get 0 coefficients
    # -> Q=0 -> guarded R~=1 -> G=0.
    beta = np.where(cdv > 0, cdv ** -0.5, 0.0)

    Px = P[:, :, 0]  # (ne, 3)
    Py = P[:, :, 1]
    dPx = Px - np.roll(Px, -1, axis=1)  # [d01, d12, d20] coefficients
    dPy = Py - np.roll(Py, -1, axis=1)

    def stack6(ax, ay):
        """rows [axh, axm, axl, ayh, aym, ayl] as bf16."""
        xh, xm, xl = _split3(ax)
        yh, ym, yl = _split3(ay)
        return np.stack([xh, xm, xl, yh, ym, yl]).astype(ml_dtypes.bfloat16)

    rhsu = np.concatenate([stack6(Px[:, v], Py[:, v]) for v in range(3)], axis=1)
    rhsd = np.concatenate(
        [
            stack6(TWO_PI * beta * dPx[:, k], TWO_PI * beta * dPy[:, k])
            for k in (0, 1)
        ],
        axis=1,
    )

    kxv = np.fft.fftfreq(RES0, d=1.0 / RES0)  # row -> freq value
    in_maps = []
    for r in range(N_CORES):
        q = np.arange(CHUNKS * 128)
        lr = q // KYPAD
        kyi = q % KYPAD
        kxrow = kxv[32 * r + lr]
        lhs = np.zeros((6, CHUNKS * 128), np.float32)
        lhs[0:3] = kxrow
        lhs[3:6] = kyi
        in_maps.append(
            {
                "lhs6": lhs.astype(ml_dtypes.bfloat16),
                "rhsu": rhsu,
                "rhsd": rhsd,
            }
        )
    return in_maps, float(np.sum(cdv))


# largest element count whose 3-plane PSUM half fits one 512-col bank
_MAX_GROUP = 170


def kernel(V, E, D, _want_trace=False):
    from concourse.bass_utils import run_bass_kernel_spmd

    V = np.asarray(V, np.float32)
    E = np.asarray(E)
    D = np.asarray(D, np.float32)

    # identical elements (same vertex-index rows) contribute identical
    # spectra scaled by their D -> deduplicate and aggregate D
    Eu, inv = np.unique(E, axis=0, return_inverse=True)
    Dagg = np.zeros((Eu.shape[0], D.shape[1]), np.float64)
    np.add.at(Dagg, inv.reshape(-1), D.astype(np.float64))
    n_eff = Eu.shape[0]
    P = V[Eu].astype(np.float64)  # (n_eff, 3, 2)

    # split into groups small enough for the PSUM layout; partial spectra
    # are linear in elements, so group results just add
    n_groups = -(-n_eff // _MAX_GROUP)
    per = -(-n_eff // n_groups)
    n_pad = max(8, -(-per // 2) * 2)
    if n_pad not in _compiled:
        _compiled[n_pad] = _build_program(n_pad)
    nc = _compiled[n_pad]

    fo_sum = [np.zeros((128, 2 * CHUNKS), np.float64) for _ in range(N_CORES)]
    cd_total = 0.0
    res = None
    for g in range(n_groups):
        sl = slice(g * per, min((g + 1) * per, n_eff))
        in_maps, cd_sum = _host_prep_group(P[sl], Dagg[sl], n_pad)
        cd_total += cd_sum
        res = run_bass_kernel_spmd(
            nc, in_maps, core_ids=list(range(N_CORES)), trace=_want_trace
        )
        for r in range(N_CORES):
            fo_sum[r] += res.results[r]["fout"]

    F = np.zeros((RES0, RES1, 1, 2), np.float32)
    for r in range(N_CORES):
        fo = fo_sum[r].astype(np.float32)  # (128, 2*CHUNKS)
        re_raw = fo[:, 0::2].T.reshape(-1)  # (33*128,) chunk-major
        im_raw = fo[:, 1::2].T.reshape(-1)
        re = re_raw.reshape(ROWS_PER_CORE, KYPAD)[:, :RES1]
        im = im_raw.reshape(ROWS_PER_CORE, KYPAD)[:, :RES1]
        F[32 * r : 32 * r + 32, :, 0, 0] = -65536.0 * re
        F[32 * r : 32 * r + 32, :, 0, 1] = 65536.0 * im
    F[0, 0, 0, :] = np.float32(32768.0 * cd_total)
    if _want_trace:
        return F, res
    return F


# revision 9
# speedup vs baseline: 1.1504x; 1.1504x over previous
"""DDSL simplex-FT Bass kernel for Trainium2 (8 NeuronCores), v2.

Math: for triangles (j=2) with vertices P[e,v,:] (from V[E]), densities D,
output spectrum F over the 256x129 rfft2 grid:

  sig_v(e,f)  = 2*pi*(kx*Px_v + ky*Py_v)
  d01=sig0-sig1, d12=sig1-sig2, d20=sig2-sig0,  Q = d01*d12*d20
  tmp_re = -(d12*cos(sig0)+d20*cos(sig1)+d01*cos(sig2))/Q   (etc. for im)
  F_raw  = sum_e CD_e * tmp;  F = -(256^2)*F_raw  (+ DC override)

v2 restructure (vs v1):
  - beta trick: host scales the d-plane coefficients by beta=cd^-1/2, so
    QR3->recip yields R~ = cd*R and G_v = d~_pair * R~ = cd*d_pair*R with
    no separate CD-premultiplied planes (gg matmuls and PSUM arena gone).
  - d01+d12+d20=0  =>  G1 = -(G0+G2), so
    sum_v G_v t_v = G0*(t0-t1) + G2*(t2-t1): no G1 plane; trig DIFF planes
    (Pool) replace the 3-plane multiply.
  - cos from the sin args: cos(x) = sin(pi/2 - |x|), |x|<=pi stays in the
    ACT Sin table range; kills the +0.25 cos matmuls and halves FRAC.
  - products in bf16 (DVE 2x mode), reduction via tensor_scalar accum_out
    in 4x mode; accumulation is fp32 in the accumulator. Host-verified
    numerics: l2 rel err ~7e-3 vs fp64 (gate 2e-2).
  - PSUM per pair: uu 2 banks + dd 2 banks, pool bufs=2 -> full
    pair-level double buffering.

Sharding: frequency rows split 8 ways (32 kx rows x 132 padded ky cols per
core = 33 chunks of 128 freqs on partitions); duplicate elements merged on
host (D aggregated), survivor count padded to n_pad on the free dim.
"""

import math
import numpy as np
import ml_dtypes

N_CORES = 8
N_ELEM = 256
RES0, RES1 = 256, 129
KYPAD = 132  # 32*132 = 4224 = 33*128
ROWS_PER_CORE = 32
CHUNKS = (ROWS_PER_CORE * KYPAD) // 128  # 33
MAGIC = float(np.float32(1.5 * 2**23))
TWO_PI = 2 * math.pi

_compiled = {}


def _split3(v):
    """3-way bf16 split of fp32/64 values: v ~= h+m+l with exact bf16 parts."""
    v32 = np.asarray(v, np.float32)
    h = v32.astype(ml_dtypes.bfloat16)
    r = (v32 - h.astype(np.float32)).astype(np.float32)
    m = r.astype(ml_dtypes.bfloat16)
    l = (r - m.astype(np.float32)).astype(ml_dtypes.bfloat16)
    return h, m, l


def _register_ops():
    import concourse.dve_ops as dve_ops_mod
    from concourse.dve_ops import DveOp, OPS
    from concourse.dve_spec import (
        Spec,
        Src0,
        Src1,
        C0,
        C1,
        One,
        Zero,
        eq,
        select,
        lower as dve_lower,
        _has_src1 as has_src1,
    )
    from concourse.dve_uop import DveOpSpec

    def register_op(name, spec, subdim=False):
        existing = {op.name: op for op in OPS}
        if name in existing:
            return existing[name]
        opcode = dve_ops_mod._CUSTOM_DVE_ROW_BASE + len(OPS)
        assert opcode < 0x20
        dve_ops_mod._SUB_OPCODE_FOR_NAME[name] = opcode
        shas = {}
        for ver in ("v3",):
            uops = dve_lower(spec, ver=ver)
            shas[ver] = DveOpSpec(
                name=name, opcode=opcode, uops=uops, rd1_en=has_src1(spec)
            ).sha(ver)
        op = DveOp(name, spec, subdim=subdim, uops_sha=shas)
        OPS.append(op)
        dve_ops_mod.CUSTOM_DVE_SPECS[name] = spec
        return op

    frac = register_op("FRAC_SCALED", Spec(body=(Src0 - ((Src0 + C0) - C0)) * C1))
    _q = Src0 * Src1 * (Src0 + Src1)
    qr3 = register_op("QR3_GUARD", Spec(body=select(eq(_q, Zero), One, _q)))

    # G = Src1 / Src0 via BITWISE_NOT exponent-flip seed + ONE Newton pass
    # (max rel err 1.7e-3 < the bf16 rounding already in the chain); fusing
    # the divide into the d-plane multiply removes a separate reciprocal op
    # and the Pool-side G0 multiply.
    from concourse.dve_spec import Bin as SBin, AluOp as SAluOp

    def _ref_recip1_mul(in0, in1, c0, c1, c2):
        not_x = (~in0.view(np.int32)).view(np.float32)
        y0 = (not_x * np.float32(c0)).astype(np.float32)
        y1 = (y0 * (np.float32(c1) - in0 * y0)).astype(np.float32)
        return (y1 * in1).astype(np.float32)

    _y0 = SBin(SAluOp.BITWISE_NOT, Src0, Src0) * C0
    _y1 = _y0 * (C1 - Src0 * _y0)
    rcpm = register_op(
        "RECIP1_MUL", Spec(body=_y1 * Src1, reference=_ref_recip1_mul)
    )
    return frac, qr3, rcpm


def _build_program(n_pad):
    import concourse.bacc as bacc
    import concourse.mybir as mybir
    from concourse.tile import TileContext

    FRAC, QR3, RCPM = _register_ops()
    from concourse.dve_ops import RECIP_APPROX_FAST_CONSTS

    RC0 = RECIP_APPROX_FAST_CONSTS["s0"]
    RC1 = RECIP_APPROX_FAST_CONSTS["s1"]

    f32 = mybir.dt.float32
    bf16 = mybir.dt.bfloat16
    nc = bacc.Bacc("TRN2", target_bir_lowering=False)

    lhs_d = nc.dram_tensor("lhs6", [6, CHUNKS * 128], bf16, kind="ExternalInput")
    rhsu_d = nc.dram_tensor("rhsu", [6, 3 * n_pad], bf16, kind="ExternalInput")
    rhsd_d = nc.dram_tensor("rhsd", [6, 2 * n_pad], bf16, kind="ExternalInput")
    fout_d = nc.dram_tensor("fout", [128, 2 * CHUNKS], f32, kind="ExternalOutput")

    E = n_pad
    EB = 3 * E
    Sin = mybir.ActivationFunctionType.Sin
    Abs = mybir.ActivationFunctionType.Abs
    Copy = mybir.ActivationFunctionType.Copy
    mult = mybir.AluOpType.mult
    add = mybir.AluOpType.add
    HB = 512  # psum half stride (cols); one 2KB bank

    # matmul outputs must stay inside one PSUM bank per chunk-half
    assert 3 * E <= HB and 2 * E <= HB, f"bad n_pad {E}"

    with TileContext(nc) as tc:
        with (
            tc.tile_pool(name="const", bufs=1) as cpool,
            tc.tile_pool(name="work", bufs=4) as pool,
            tc.tile_pool(name="psum", bufs=2, space="PSUM") as psp,
        ):
            lhs = cpool.tile([6, CHUNKS * 128], bf16)
            rhsu = cpool.tile([6, 3 * E], bf16)
            rhsd = cpool.tile([6, 2 * E], bf16)
            fout = cpool.tile([128, 2 * CHUNKS], f32)
            pi2 = cpool.tile([128, 1], f32)
            nc.gpsimd.memset(pi2[:], math.pi / 2)
            nc.sync.dma_start(lhs[:], lhs_d[:])
            nc.sync.dma_start(rhsu[:], rhsu_d[:])
            nc.sync.dma_start(rhsd[:], rhsd_d[:])

            # Pairs of chunks flow through a software-pipelined 3-stage
            # schedule: P(i) matmuls + FRAC + QR + G-planes (PSUM-coupled),
            # T(i-1) trig + diff planes, C(i-2) products + accumulation.
            # The lag-2 consume hides the ACT trig -> Pool diff latency
            # chain behind two full iterations of DVE work.
            pairs = [
                [2 * p, 2 * p + 1] if 2 * p + 1 < CHUNKS else [2 * p]
                for p in range((CHUNKS + 1) // 2)
            ]
            cd = nc.vector._custom_dve

            def blk(ap, off, width, stride):
                """(128, nblk, width) view of a compact tile."""
                return ap.rearrange("p (t x) -> p t x", x=stride)[
                    :, :, off : off + width
                ]

            def produce(pc):
                T = len(pc)
                uu = psp.tile([128, T * HB], f32, tag="uu")
                dd = psp.tile([128, T * HB], f32, tag="dd")
                mm = nc.tensor.matmul
                for h, c in enumerate(pc):
                    l6 = lhs[:, c * 128 : (c + 1) * 128]
                    b = h * HB
                    for v in range(3):
                        mm(uu[:, b + v * E : b + (v + 1) * E], l6,
                           rhsu[:, v * E : (v + 1) * E], start=True, stop=True)
                    mm(dd[:, b : b + E], l6, rhsd[:, 0:E], start=True, stop=True)
                    mm(dd[:, b + E : b + 2 * E], l6, rhsd[:, E : 2 * E],
                       start=True, stop=True)

                def pblk(ap, off, width):
                    return ap.rearrange("p (t x) -> p t x", x=HB)[
                        :, :, off : off + width
                    ]

                # d12 PSUM->SBUF (QR3 may keep at most one PSUM operand)
                d12s = pool.tile([128, T * E], f32, tag="d12s")
                nc.scalar.activation(blk(d12s[:], 0, E, E), pblk(dd[:], E, E),
                                     Copy)
                # FRAC: arg = 2*pi*(u - round(u)) in [-pi, pi]
                arg = pool.tile([128, T * EB], f32, tag="arg")
                cd(FRAC, out=blk(arg[:], 0, EB, EB), in0=pblk(uu[:], 0, EB),
                   s0=MAGIC, s1=TWO_PI)
                # -Q~ = d01*d12*(d01+d12), zero-guarded
                mQ = pool.tile([128, T * E], f32, tag="mQ")
                cd(QR3, out=blk(mQ[:], 0, E, E), in0=blk(d12s[:], 0, E, E),
                   in1=pblk(dd[:], 0, E))
                # G planes (bf16): per chunk [G0|G2] = [d12~|d01~]/(-Q~);
                # fused seed+1NR divide-multiply, one op per plane
                Gt = pool.tile([128, T * 2 * E], bf16, tag="Gt")
                cd(RCPM, out=blk(Gt[:], 0, E, 2 * E),
                   in0=blk(mQ[:], 0, E, E), in1=blk(d12s[:], 0, E, E),
                   s0=RC0, s1=RC1)
                cd(RCPM, out=blk(Gt[:], E, E, 2 * E),
                   in0=blk(mQ[:], 0, E, E), in1=pblk(dd[:], 0, E),
                   s0=RC0, s1=RC1)
                return {"pc": pc, "T": T, "arg": arg, "Gt": Gt}

            def trig(st):
                T, arg = st["T"], st["arg"]
                # trig planes (bf16): sin(arg); cos(arg) = sin(pi/2 - |arg|)
                tr_s = pool.tile([128, T * EB], bf16, tag="tr_s")
                nc.scalar.activation(tr_s[:], arg[:], Sin)
                ab = pool.tile([128, T * EB], f32, tag="ab")
                nc.scalar.activation(ab[:], arg[:], Abs)
                tr_c = pool.tile([128, T * EB], bf16, tag="tr_c")
                nc.scalar.activation(tr_c[:], ab[:], Sin, bias=pi2[:],
                                     scale=-1.0)
                # trig diff planes (bf16, Pool): per chunk [t0-t1 | t2-t1]
                sd = pool.tile([128, T * 2 * E], bf16, tag="sd")
                cdf = pool.tile([128, T * 2 * E], bf16, tag="cdf")
                for src, dst in ((tr_s, sd), (tr_c, cdf)):
                    nc.gpsimd.tensor_sub(
                        blk(dst[:], 0, E, 2 * E), blk(src[:], 0, E, EB),
                        blk(src[:], E, E, EB),
                    )
                    nc.gpsimd.tensor_sub(
                        blk(dst[:], E, E, 2 * E), blk(src[:], 2 * E, E, EB),
                        blk(src[:], E, E, EB),
                    )
                st["sd"], st["cdf"] = sd, cdf

            def consume(st):
                # products (bf16, DVE 2x); per-chunk fp32 accumulation via
                # tensor_scalar 4x with accum_out straight into fout
                pc, Gt = st["pc"], st["Gt"]
                scr = pool.tile([128, 4 * 2 * E], bf16, tag="scr")
                for k, dif in enumerate((st["sd"], st["cdf"])):
                    pr = pool.tile([128, st["T"] * 2 * E], bf16, tag=f"pr{k}")
                    nc.vector.tensor_mul(pr[:], Gt[:], dif[:])
                    for h, c in enumerate(pc):
                        nc.vector.tensor_scalar(
                            out=scr[:, (2 * h + k) * 2 * E
                                    : (2 * h + k + 1) * 2 * E],
                            in0=pr[:, h * 2 * E : (h + 1) * 2 * E],
                            scalar1=1.0, scalar2=0.0, op0=mult, op1=add,
                            accum_out=fout[:, 2 * c + (1 - k)
                                           : 2 * c + (1 - k) + 1],
                        )

            state = []
            for pc in pairs:
                state.append(produce(pc))
                if len(state) >= 2:
                    trig(state[-2])
                if len(state) >= 3:
                    consume(state[-3])
            trig(state[-1])
            consume(state[-2])
            consume(state[-1])

            nc.sync.dma_start(fout_d[:], fout[:])

    nc.compile()
    return nc


def _host_prep_group(P, Dagg, n_pad):
    """Build per-core input maps for one padded element group."""
    n_eff = P.shape[0]
    # pad with copies of element 0 carrying zero density (zero contribution)
    if n_pad > n_eff:
        P = np.concatenate([P, np.repeat(P[:1], n_pad - n_eff, axis=0)], axis=0)
        Dagg = np.concatenate(
            [Dagg, np.zeros((n_pad - n_eff, Dagg.shape[1]))], axis=0
        )
    ne = n_pad

    # CD = 2 * area * D via Cayley-Menger (matches reference up to fp rounding)
    D2 = ((P[:, :, None, :] - P[:, None, :, :]) ** 2).sum(-1)
    B = np.ones((ne, 4, 4))
    B[:, 0, 0] = 0.0
    B[:, 1:, 1:] = D2
    vol2 = (-1.0) / 4.0 * np.linalg.det(B) / 4.0  # ((-1)^3)/(2^2)/(2!^2)*det
    content = np.sqrt(np.clip(vol2, 0.0, None))
    CD = 2.0 * content[:, None] * Dagg  # (ne, n_ch=1)
    cdv = CD[:, 0]  # n_ch == 1

    # beta trick: scale d-plane coefficients by cd^-1/2 so G = d~*R~ carries
    # cd automatically; cd==0 (padding / zero-density) rows get 0 coefficients
    # -> Q=0 -> guarded R~=1 -> G=0.
    beta = np.where(cdv > 0, cdv ** -0.5, 0.0)

    Px = P[:, :, 0]  # (ne, 3)
    Py = P[:, :, 1]
    dPx = Px - np.roll(Px, -1, axis=1)  # [d01, d12, d20] coefficients
    dPy = Py - np.roll(Py, -1, axis=1)

    def stack6(ax, ay):
        """rows [axh, axm, axl, ayh, aym, ayl] as bf16."""
        xh, xm, xl = _split3(ax)
        yh, ym, yl = _split3(ay)
        return np.stack([xh, xm, xl, yh, ym, yl]).astype(ml_dtypes.bfloat16)

    rhsu = np.concatenate([stack6(Px[:, v], Py[:, v]) for v in range(3)], axis=1)
    rhsd = np.concatenate(
        [
            stack6(TWO_PI * beta * dPx[:, k], TWO_PI * beta * dPy[:, k])
            for k in (0, 1)
        ],
        axis=1,
    )

    kxv = np.fft.fftfreq(RES0, d=1.0 / RES0)  # row -> freq value
    in_maps = []
    for r in range(N_CORES):
        q = np.arange(CHUNKS * 128)
        lr = q // KYPAD
        kyi = q % KYPAD
        kxrow = kxv[32 * r + lr]
        lhs = np.zeros((6, CHUNKS * 128), np.float32)
        lhs[0:3] = kxrow
        lhs[3:6] = kyi
        in_maps.append(
            {
                "lhs6": lhs.astype(ml_dtypes.bfloat16),
                "rhsu": rhsu,
                "rhsd": rhsd,
            }
        )
    return in_maps, float(np.sum(cdv))


# largest element count whose 3-plane PSUM half fits one 512-col bank
_MAX_GROUP = 170


def kernel(V, E, D, _want_trace=False):
    from concourse.bass_utils import run_bass_kernel_spmd

    V = np.asarray(V, np.float32)
    E = np.asarray(E)
    D = np.asarray(D, np.float32)

    # identical elements (same vertex-index rows) contribute identical
    # spectra scaled by their D -> deduplicate and aggregate D
    Eu, inv = np.unique(E, axis=0, return_inverse=True)
    Dagg = np.zeros((Eu.shape[0], D.shape[1]), np.float64)
    np.add.at(Dagg, inv.reshape(-1), D.astype(np.float64))
    n_eff = Eu.shape[0]
    P = V[Eu].astype(np.float64)  # (n_eff, 3, 2)

    # split into groups small enough for the PSUM layout; partial spectra
    # are linear in elements, so group results just add
    n_groups = -(-n_eff // _MAX_GROUP)
    per = -(-n_eff // n_groups)
    n_pad = max(8, -(-per // 2) * 2)
    if n_pad not in _compiled:
        _compiled[n_pad] = _build_program(n_pad)
    nc = _compiled[n_pad]

    fo_sum = [np.zeros((128, 2 * CHUNKS), np.float64) for _ in range(N_CORES)]
    cd_total = 0.0
    res = None
    for g in range(n_groups):
        sl = slice(g * per, min((g + 1) * per, n_eff))
        in_maps, cd_sum = _host_prep_group(P[sl], Dagg[sl], n_pad)
        cd_total += cd_sum
        res = run_bass_kernel_spmd(
            nc, in_maps, core_ids=list(range(N_CORES)), trace=_want_trace
        )
        for r in range(N_CORES):
            fo_sum[r] += res.results[r]["fout"]

    F = np.zeros((RES0, RES1, 1, 2), np.float32)
    for r in range(N_CORES):
        fo = fo_sum[r].astype(np.float32)  # (128, 2*CHUNKS)
        re_raw = fo[:, 0::2].T.reshape(-1)  # (33*128,) chunk-major
        im_raw = fo[:, 1::2].T.reshape(-1)
        re = re_raw.reshape(ROWS_PER_CORE, KYPAD)[:, :RES1]
        im = im_raw.reshape(ROWS_PER_CORE, KYPAD)[:, :RES1]
        F[32 * r : 32 * r + 32, :, 0, 0] = -65536.0 * re
        F[32 * r : 32 * r + 32, :, 0, 1] = 65536.0 * im
    F[0, 0, 0, :] = np.float32(32768.0 * cd_total)
    if _want_trace:
        return F, res
    return F
